# revision 1
# baseline (speedup 1.0000x reference)
"""Trainium2 Bass kernel for a 2-layer edge-featured GAT + mean-pool + FC.

Sharding: 256 graphs are split 32-per-core across 8 cores. Because `batch` is
sorted, each core owns a contiguous node range (graph-aligned), so both the
per-destination softmax segments and the mean-pool segments are core-local.
Edges are assigned to the core that owns their destination node. Between
layers, per-core node tables (features + attention logit terms) are
AllGathered so every core can gather arbitrary source rows.

Edge phase (per layer): edges sorted by dst are packed into 128-edge chunks
grouped by destination node tile (128 nodes). Per chunk, a dma_gather pulls
the source-node table rows [h | a_src | a_dst | pad], a second (narrow)
dma_gather pulls the destination rows' logit slice. Attention weights are
p = exp(leaky_relu(a_src+a_dst+w*q)) computed as max(exp(x), exp(0.2x)),
messages are h*p, and the segment-sum over destinations is a one-hot matmul
accumulated in PSUM — with p itself carried as extra columns to produce the
softmax denominators. Softmax normalization happens once per node after
aggregation: out = (sum p*h) / (sum p + 1e-16), exactly equivalent to the
reference's per-edge normalization (the max-subtraction cancels in the
ratio). Pad edge slots carry dst_local = -1 so their one-hot columns are all
zero and they contribute nothing.
"""

import sys

sys.path.insert(0, "/opt/trn_rl_repo")

import math
from contextlib import ExitStack

import numpy as np

import concourse.bacc as bacc
import concourse.bass as bass
import concourse.mybir as mybir
import concourse.tile as tile
from concourse.bass_utils import run_bass_kernel_spmd
from concourse.masks import make_identity

P = 128
NCORES = 8
SP = False  # dma_gather single_packet
SKIP = set()  # benchmarking ablations: gsrc, gdst, msg, mm

FULL_CFG = dict(N=20000, E=640000, FIN=128, HID=64, HEADS=4, NG=256, OUT=32)

F32 = mybir.dt.float32
I16 = mybir.dt.int16


# ---------------------------------------------------------------------------
# Host-side preparation: integer index manipulation + array reordering only.
# ---------------------------------------------------------------------------
def prepare(inputs, cfg):
    N, E, FIN, HID, HEADS, NG, OUT = (
        cfg["N"], cfg["E"], cfg["FIN"], cfg["HID"], cfg["HEADS"], cfg["NG"],
        cfg["OUT"],
    )
    GPC = NG // NCORES  # graphs per core

    x = np.asarray(inputs["x"], np.float32)
    ei = np.asarray(inputs["edge_index"], np.int64)
    ea = np.asarray(inputs["edge_attr"], np.float32)
    batch = np.asarray(inputs["batch"], np.int64)
    src, dst = ei[0], ei[1]

    # node ranges per core (graph-aligned; batch is sorted)
    bounds = np.searchsorted(batch, np.arange(NCORES + 1) * GPC)
    node_cnt = np.diff(bounds)
    NT = max(1, math.ceil(node_cnt.max() / P))
    NSLICE = NT * P
    NROWS = NCORES * NSLICE
    assert NROWS < 32768, f"int16 gather index overflow: {NROWS}"

    core_of_node = np.minimum(batch // GPC, NCORES - 1).astype(np.int64)
    rowid = np.empty(N, np.int64)
    for c in range(NCORES):
        ns, ne = bounds[c], bounds[c + 1]
        rowid[ns:ne] = c * NSLICE + np.arange(ne - ns)

    # edges sorted by dst; since batch is sorted, core blocks are contiguous
    order = np.argsort(dst, kind="stable")
    dsts = dst[order]
    srcs = src[order]
    ws = ea[order, 0]
    ecore = core_of_node[dsts]
    ebounds = np.searchsorted(ecore, np.arange(NCORES + 1))

    # chunks-per-tile: max over all (core, tile), rounded up to even
    cpt_max = 1
    tile_edge_counts = []
    for c in range(NCORES):
        es, ee = ebounds[c], ebounds[c + 1]
        dln = dsts[es:ee] - bounds[c]
        tid = dln // P
        cnts = np.bincount(tid, minlength=NT)
        tile_edge_counts.append(cnts)
        if len(cnts):
            cpt_max = max(cpt_max, math.ceil(cnts.max() / P))
    CPT = cpt_max + (cpt_max % 2)  # even
    CPT = max(CPT, 2)
    CH = CPT // 2
    NCHUNK = NT * CPT

    per_core = []
    for c in range(NCORES):
        ns, ne = bounds[c], bounds[c + 1]
        es, ee = ebounds[c], ebounds[c + 1]
        nloc = ne - ns

        xs = np.zeros((NSLICE, FIN), np.float32)
        xs[:nloc] = x[ns:ne]

        gl = np.full((NT * P,), -1.0, np.float32)
        gl[:nloc] = (batch[ns:ne] - c * GPC).astype(np.float32)
        gl_dev = gl.reshape(NT, P).T.copy()  # [128, NT]

        srcrow = np.zeros((NT, CPT * P), np.int64)
        dstrow = np.zeros((NT, CPT * P), np.int64)
        dstl = np.full((NT, CPT * P), -1.0, np.float32)
        wv = np.zeros((NT, CPT * P), np.float32)

        dln = dsts[es:ee] - ns
        tid = dln // P
        cnts = tile_edge_counts[c]
        off = np.zeros(NT + 1, np.int64)
        off[1:NT + 1] = np.cumsum(cnts[:NT])
        for t in range(NT):
            k = int(cnts[t]) if t < len(cnts) else 0
            if k == 0:
                continue
            sel = slice(es + int(off[t]), es + int(off[t]) + k)
            srcrow[t, :k] = rowid[srcs[sel]]
            dstrow[t, :k] = rowid[dsts[sel]]
            dstl[t, :k] = (dln[int(off[t]):int(off[t]) + k] % P).astype(
                np.float32)
            wv[t, :k] = ws[sel]

        # device layouts
        dstl_dev = dstl.reshape(NCHUNK, P).T.copy()       # [128, NCHUNK]
        wv_dev = wv.reshape(NCHUNK, P).T.copy()

        def wrap_idx(arr):  # [NT, CPT*P] -> [128, NT*CPT*8] int16
            blocks = []
            for t in range(NT):
                for h in range(2):
                    ids = arr[t, h * CH * P:(h + 1) * CH * P]
                    a = ids.reshape(CH * 8, 16).T  # [16, CH*8]
                    blocks.append(np.tile(a, (8, 1)))
            return np.ascontiguousarray(
                np.concatenate(blocks, axis=1)).astype(np.int16)

        per_core.append(dict(
            xs=xs, gl=gl_dev, dstl=dstl_dev, wv=wv_dev,
            idxs=wrap_idx(srcrow), idxd=wrap_idx(dstrow),
        ))

    # weight-side constants (tiny, host-replicated)
    W1 = np.asarray(inputs["W1"], np.float32)            # [FIN, H*HID]
    W2 = np.asarray(inputs["W2"], np.float32)            # [H*HID, HID]
    as1 = np.asarray(inputs["att_src1"], np.float32).reshape(-1)
    ad1 = np.asarray(inputs["att_dst1"], np.float32).reshape(-1)
    as2 = np.asarray(inputs["att_src2"], np.float32).reshape(-1)
    ad2 = np.asarray(inputs["att_dst2"], np.float32).reshape(-1)
    q1 = (np.asarray(inputs["We1"], np.float32).reshape(HEADS, HID)
          * np.asarray(inputs["att_edge1"], np.float32)).sum(axis=1)  # [H]
    q2 = float((np.asarray(inputs["We2"], np.float32).reshape(-1)
                * np.asarray(inputs["att_edge2"], np.float32).reshape(-1))
               .sum())
    b1 = np.asarray(inputs["b1"], np.float32)
    b2 = np.asarray(inputs["b2"], np.float32)
    fcW = np.asarray(inputs["fcW"], np.float32)
    fcb = np.asarray(inputs["fcb"], np.float32)

    rep = lambda vv: np.tile(vv[None, :].astype(np.float32), (P, 1)).copy()
    consts = dict(
        W1=W1, W2=W2,
        as1b=rep(as1), ad1b=rep(ad1), b1b=rep(b1),
        as2b=rep(as2), ad2b=rep(ad2), b2b=rep(b2),
        q1b=rep(q1), q2b=np.full((P, 1), q2, np.float32),
        fcw=fcW, fcbb=rep(fcb),
        iota=np.tile(np.arange(P, dtype=np.float32)[None, :], (P, 1)).copy(),
    )

    in_maps = []
    for c in range(NCORES):
        m = dict(per_core[c])
        m.update(consts)
        in_maps.append(m)

    meta = dict(NT=NT, CPT=CPT, CH=CH, NSLICE=NSLICE, NROWS=NROWS,
                GPC=GPC, **cfg)
    return in_maps, meta


# ---------------------------------------------------------------------------
# Device program.
# ---------------------------------------------------------------------------
def build(meta, reps=1, num_devices=NCORES):
    NT, CPT, CH = meta["NT"], meta["CPT"], meta["CH"]
    NSLICE, NROWS, GPC = meta["NSLICE"], meta["NROWS"], meta["GPC"]
    FIN, HID, HEADS, OUT = meta["FIN"], meta["HID"], meta["HEADS"], meta["OUT"]
    D1 = HEADS * HID          # 256
    ROW1 = D1 + 64            # 320 floats: h(256) asrc(4@256) adst(4@260) pad
    OFF1 = D1                 # dst-slice offset (floats)
    ROW2 = 2 * HID            # 128 floats: h2(64) asrc2(@64) adst2(@65) pad
    OFF2 = HID
    NI = CH * P               # idxs per gather group
    NIc = NI // 16            # idx columns per group
    NCHUNK = NT * CPT
    A = mybir.AluOpType
    ACT = mybir.ActivationFunctionType
    X = mybir.AxisListType.X
    rg = [list(range(NCORES))]

    nc = bacc.Bacc("TRN2", target_bir_lowering=False, debug=False,
                   num_devices=num_devices,
                   dynamic_dma_scratch_size=65536)

    def din(name, shape, dtype=F32):
        return nc.dram_tensor(name, list(shape), dtype,
                              kind="ExternalInput").ap()

    xs = din("xs", (NSLICE, FIN))
    idxs_d = din("idxs", (P, NT * CPT * 8), I16)
    idxd_d = din("idxd", (P, NT * CPT * 8), I16)
    dstl_d = din("dstl", (P, NCHUNK))
    wv_d = din("wv", (P, NCHUNK))
    gl_d = din("gl", (P, NT))
    W1_d = din("W1", (FIN, D1))
    W2_d = din("W2", (D1, HID))
    as1_d = din("as1b", (P, D1))
    ad1_d = din("ad1b", (P, D1))
    b1_d = din("b1b", (P, D1))
    as2_d = din("as2b", (P, HID))
    ad2_d = din("ad2b", (P, HID))
    b2_d = din("b2b", (P, HID))
    q1_d = din("q1b", (P, HEADS))
    q2_d = din("q2b", (P, 1))
    fcw_d = din("fcw", (HID, OUT))
    fcb_d = din("fcbb", (P, OUT))
    iota_d = din("iota", (P, P))

    out_d = nc.dram_tensor("out", [GPC, OUT], F32, kind="ExternalOutput").ap()

    with tile.TileContext(nc) as tc, ExitStack() as st:
        constp = st.enter_context(tc.tile_pool(name="constp", bufs=1))
        drp = st.enter_context(tc.tile_pool(name="drp", bufs=1, space="DRAM"))

        # whole-kernel constants
        iota_sb = constp.tile([P, P], F32)
        nc.sync.dma_start(iota_sb[:], iota_d[:])
        ident = constp.tile([P, P], F32)
        make_identity(nc, ident[:])
        dstl_sb = constp.tile([P, NCHUNK], F32)
        nc.sync.dma_start(dstl_sb[:], dstl_d[:])
        wv_sb = constp.tile([P, NCHUNK], F32)
        nc.sync.dma_start(wv_sb[:], wv_d[:])
        gl_sb = constp.tile([P, NT], F32)
        nc.sync.dma_start(gl_sb[:], gl_d[:])
        q1_sb = constp.tile([P, HEADS], F32)
        nc.sync.dma_start(q1_sb[:], q1_d[:])
        q2_sb = constp.tile([P, 1], F32)
        nc.sync.dma_start(q2_sb[:], q2_d[:])
        ixs_all = constp.tile([P, NT * CPT * 8], I16)
        nc.sync.dma_start(ixs_all[:], idxs_d[:])
        ixd_all = constp.tile([P, NT * CPT * 8], I16)
        nc.sync.dma_start(ixd_all[:], idxd_d[:])

        # repetition loop (reps>1 only for benchmarking)
        for _rep in range(reps):
            t1loc = drp.tile([NSLICE, ROW1], F32, name=f"t1loc{_rep}")
            t1full = drp.tile([NROWS, ROW1], F32, addr_space="Shared",
                              name=f"t1full{_rep}")
            t2loc = drp.tile([NSLICE, ROW2], F32, name=f"t2loc{_rep}")
            t2full = drp.tile([NROWS, ROW2], F32, addr_space="Shared",
                              name=f"t2full{_rep}")

            # ---------------- Phase 0: h1 = x @ W1, a_src/a_dst, table1 ---------
            with tc.tile_pool(name="ph0", bufs=1) as sp, \
                 tc.tile_pool(name="ph0b", bufs=2) as sp2, \
                 tc.tile_pool(name="ph0p", bufs=2, space="PSUM") as pp:
                w1_sb = sp.tile([P, D1], F32)
                nc.sync.dma_start(w1_sb[:], W1_d[:])
                as1_sb = sp.tile([P, D1], F32)
                nc.sync.dma_start(as1_sb[:], as1_d[:])
                ad1_sb = sp.tile([P, D1], F32)
                nc.sync.dma_start(ad1_sb[:], ad1_d[:])
                xall = sp.tile([P, NT, FIN], F32)
                nc.sync.dma_start(xall[:],
                                  xs[:].rearrange("(t p) f -> p t f", p=P))
                for t in range(NT if "ph0" not in SKIP else 0):
                    xT_ps = pp.tile([P, P], F32, space="PSUM")
                    nc.tensor.transpose(xT_ps[:], xall[:, t, :], ident[:])
                    xT = sp2.tile([P, P], F32)
                    nc.vector.tensor_copy(out=xT[:], in_=xT_ps[:])
                    h_ps = pp.tile([P, D1], F32, space="PSUM")
                    nc.tensor.matmul(h_ps[:], lhsT=xT[:], rhs=w1_sb[:],
                                     start=True, stop=True)
                    t1t = sp2.tile([P, ROW1], F32)
                    tmp = sp2.tile([P, D1], F32)
                    nc.vector.tensor_tensor(out=tmp[:], in0=h_ps[:],
                                            in1=as1_sb[:], op=A.mult)
                    nc.vector.tensor_reduce(
                        out=t1t[:, D1:D1 + HEADS],
                        in_=tmp[:].rearrange("p (h f) -> p h f", h=HEADS),
                        axis=X, op=A.add)
                    nc.vector.tensor_tensor(out=tmp[:], in0=h_ps[:],
                                            in1=ad1_sb[:], op=A.mult)
                    nc.vector.tensor_reduce(
                        out=t1t[:, D1 + HEADS:D1 + 2 * HEADS],
                        in_=tmp[:].rearrange("p (h f) -> p h f", h=HEADS),
                        axis=X, op=A.add)
                    nc.vector.tensor_copy(out=t1t[:, 0:D1], in_=h_ps[:])
                    nc.vector.memset(t1t[:, D1 + 2 * HEADS:ROW1], 0.0)
                    nc.sync.dma_start(t1loc[t * P:(t + 1) * P, :], t1t[:])
                if "ag" not in SKIP:
                    nc.gpsimd.collective_compute(
                        "AllGather", A.bypass, replica_groups=rg,
                        ins=[t1loc[:]], outs=[t1full[:]])

            # ---------------- Phase 1: layer-1 edge phase -----------------------
            with tc.tile_pool(name=f"outp{_rep}", bufs=1) as outp:
              out1 = outp.tile([P, NT * D1], F32, name=f"out1_{_rep}")
              if "ph1" in SKIP:
                  nc.vector.memset(out1[:], 0.0)
              with tc.tile_pool(name="p1g", bufs=2) as pg, \
                   tc.tile_pool(name="p1gd", bufs=2) as pgd, \
                   tc.tile_pool(name="p1i", bufs=3) as pi, \
                   tc.tile_pool(name="p1w", bufs=2) as pw, \
                   tc.tile_pool(name="p1oh", bufs=2) as poh, \
                   tc.tile_pool(name="p1ps", bufs=3, space="PSUM") as pps:
                  for t in range(NT if "ph1" not in SKIP else 0):
                      acc = pps.tile([P, D1 + HEADS], F32, space="PSUM")
                      for hh in range(2):
                          gbase = (t * 2 + hh) * NIc
                          cbase = (t * 2 + hh) * CH
                          G = pg.tile([P, CH, ROW1], F32)
                          if "gsrc" not in SKIP:
                              nc.gpsimd.dma_gather(
                                  G[:], t1full[:],
                                  ixs_all[:, gbase:gbase + NIc],
                                  NI, NI, ROW1, single_packet=SP)
                          Gd = pgd.tile([P, CH, 64], F32)
                          asr = G[:, :, D1:D1 + HEADS]
                          if "gdst" not in SKIP:
                              nc.gpsimd.dma_gather(
                                  Gd[:], t1full[:, OFF1:OFF1 + 64],
                                  ixd_all[:, gbase:gbase + NIc],
                                  NI, NI, 64, elem_step=ROW1, single_packet=SP)
                              nc.vector.tensor_tensor(
                                  out=asr, in0=asr,
                                  in1=Gd[:, :, HEADS:2 * HEADS], op=A.add)
                          if "msg" not in SKIP:
                              ae = pw.tile([P, CH, HEADS], F32)
                              w_b = wv_sb[:, cbase:cbase + CH].unsqueeze(2) \
                                  .to_broadcast([P, CH, HEADS])
                              q_b = q1_sb[:].unsqueeze(1).to_broadcast([P, CH, HEADS])
                              nc.vector.tensor_tensor(out=ae[:], in0=w_b, in1=q_b,
                                                      op=A.mult)
                              nc.vector.tensor_tensor(out=asr, in0=asr, in1=ae[:],
                                                      op=A.add)
                              e2 = pw.tile([P, CH, HEADS], F32)
                              nc.scalar.activation(out=e2[:], in_=asr, func=ACT.Exp,
                                                   scale=0.2)
                              nc.scalar.activation(out=asr, in_=asr, func=ACT.Exp)
                              nc.vector.tensor_tensor(out=asr, in0=asr, in1=e2[:],
                                                      op=A.max)
                              gm = G[:, :, 0:D1].rearrange("p c (h f) -> p c h f",
                                                               h=HEADS)
                              p_b = asr.unsqueeze(3).to_broadcast([P, CH, HEADS, HID])
                              nc.vector.tensor_tensor(out=gm, in0=gm, in1=p_b,
                                                      op=A.mult)
                          if "mm" not in SKIP:
                              oh = poh.tile([P, CH, P], F32)
                              nc.vector.tensor_tensor(
                                  out=oh[:],
                                  in0=iota_sb[:].unsqueeze(1)
                                      .to_broadcast([P, CH, P]),
                                  in1=dstl_sb[:, cbase:cbase + CH]
                                      .unsqueeze(2).to_broadcast([P, CH, P]),
                                  op=A.is_equal)
                              for c in range(CH):
                                  nc.tensor.matmul(
                                      acc[:], lhsT=oh[:, c, :],
                                      rhs=G[:, c, 0:D1 + HEADS],
                                      start=(hh == 0 and c == 0),
                                      stop=(hh == 1 and c == CH - 1))
                      if "epi" not in SKIP:
                          # epilogue -> out1 tile block (softmax denominator division)
                          dn = pw.tile([P, HEADS], F32)
                          nc.vector.tensor_scalar(out=dn[:], in0=acc[:, D1:D1 + HEADS],
                                                  scalar1=1e-16, scalar2=None,
                                                  op0=A.add)
                          rc = pw.tile([P, HEADS], F32)
                          nc.vector.reciprocal(rc[:], dn[:])
                          ob = out1[:, t * D1:(t + 1) * D1]
                          nc.vector.tensor_tensor(
                              out=ob.rearrange("p (h f) -> p h f", h=HEADS),
                              in0=acc[:, 0:D1].rearrange("p (h f) -> p h f", h=HEADS),
                              in1=rc[:].unsqueeze(2).to_broadcast([P, HEADS, HID]),
                              op=A.mult)

              with tc.tile_pool(name="p1e", bufs=1) as pe:
                  b1_sb = pe.tile([P, D1], F32)
                  nc.sync.dma_start(b1_sb[:], b1_d[:])
                  if "epi" in SKIP and "ph1" not in SKIP:
                      nc.vector.memset(out1[:], 0.0)
                  for t in range(NT if ("ph1" not in SKIP and "epi" not in SKIP) else 0):
                      ob = out1[:, t * D1:(t + 1) * D1]
                      nc.vector.tensor_tensor(out=ob, in0=ob, in1=b1_sb[:],
                                              op=A.add)
                      nc.vector.tensor_scalar(out=ob, in0=ob, scalar1=0.0,
                                              scalar2=None, op0=A.max)

              # ---------------- Phase 2: h2 = relu(out1) @ W2, table2 -------------
              with tc.tile_pool(name="ph2", bufs=1) as sp, \
                   tc.tile_pool(name="ph2b", bufs=2) as sp2, \
                   tc.tile_pool(name="ph2p", bufs=2, space="PSUM") as pp:
                  w2_sb = sp.tile([P, 2, HID], F32)
                  nc.sync.dma_start(w2_sb[:],
                                    W2_d[:].rearrange("(k p) n -> p k n", p=P))
                  as2_sb = sp.tile([P, HID], F32)
                  nc.sync.dma_start(as2_sb[:], as2_d[:])
                  ad2_sb = sp.tile([P, HID], F32)
                  nc.sync.dma_start(ad2_sb[:], ad2_d[:])
                  for t in range(NT if "ph2" not in SKIP else 0):
                      h2_ps = pp.tile([P, HID], F32, space="PSUM")
                      for k in range(2):
                          hT_ps = pp.tile([P, P], F32, space="PSUM")
                          nc.tensor.transpose(
                              hT_ps[:],
                              out1[:, t * D1 + k * P:t * D1 + (k + 1) * P],
                              ident[:])
                          hT = sp2.tile([P, P], F32)
                          nc.vector.tensor_copy(out=hT[:], in_=hT_ps[:])
                          nc.tensor.matmul(h2_ps[:], lhsT=hT[:],
                                           rhs=w2_sb[:, k, :],
                                           start=(k == 0), stop=(k == 1))
                      t2t = sp2.tile([P, ROW2], F32)
                      tmp = sp2.tile([P, HID], F32)
                      nc.vector.tensor_tensor(out=tmp[:], in0=h2_ps[:],
                                              in1=as2_sb[:], op=A.mult)
                      nc.vector.tensor_reduce(out=t2t[:, OFF2:OFF2 + 1],
                                              in_=tmp[:], axis=X, op=A.add)
                      nc.vector.tensor_tensor(out=tmp[:], in0=h2_ps[:],
                                              in1=ad2_sb[:], op=A.mult)
                      nc.vector.tensor_reduce(out=t2t[:, OFF2 + 1:OFF2 + 2],
                                              in_=tmp[:], axis=X, op=A.add)
                      nc.vector.tensor_copy(out=t2t[:, 0:HID], in_=h2_ps[:])
                      nc.vector.memset(t2t[:, OFF2 + 2:ROW2], 0.0)
                      nc.sync.dma_start(t2loc[t * P:(t + 1) * P, :], t2t[:])
                  if "ag" not in SKIP:
                      nc.gpsimd.collective_compute(
                          "AllGather", A.bypass, replica_groups=rg,
                          ins=[t2loc[:]], outs=[t2full[:]])

            # ---------------- Phase 3: layer-2 edge phase + pooling -------------
            with tc.tile_pool(name="p3g", bufs=2) as pg, \
                 tc.tile_pool(name="p3gd", bufs=2) as pgd, \
                 tc.tile_pool(name="p3i", bufs=3) as pi, \
                 tc.tile_pool(name="p3w", bufs=2) as pw, \
                 tc.tile_pool(name="p3oh", bufs=2) as poh, \
                 tc.tile_pool(name="p3c", bufs=1) as pc, \
                 tc.tile_pool(name="p3ps", bufs=2, space="PSUM") as pps, \
                 tc.tile_pool(name="p3pl", bufs=1, space="PSUM") as ppl:
                b2_sb = pc.tile([P, HID], F32)
                nc.sync.dma_start(b2_sb[:], b2_d[:])
                pool_ps = ppl.tile([GPC, HID + 1], F32, space="PSUM")
                if "ph3" in SKIP or "epi" in SKIP:
                    nc.vector.memset(pool_ps[:], 1.0)
                for t in range(NT if "ph3" not in SKIP else 0):
                    acc = pps.tile([P, HID + 1], F32, space="PSUM", bufs=3)
                    for hh in range(2):
                        gbase = (t * 2 + hh) * NIc
                        cbase = (t * 2 + hh) * CH
                        G = pg.tile([P, CH, ROW2], F32)
                        if "gsrc" not in SKIP:
                            nc.gpsimd.dma_gather(
                                G[:], t2full[:],
                                ixs_all[:, gbase:gbase + NIc],
                                NI, NI, ROW2, single_packet=SP)
                        Gd = pgd.tile([P, CH, 64], F32)
                        asr = G[:, :, OFF2:OFF2 + 1]
                        if "gdst" not in SKIP:
                            nc.gpsimd.dma_gather(
                                Gd[:], t2full[:, OFF2:OFF2 + 64],
                                ixd_all[:, gbase:gbase + NIc],
                                NI, NI, 64, elem_step=ROW2, single_packet=SP)
                            nc.vector.tensor_tensor(out=asr, in0=asr,
                                                    in1=Gd[:, :, 1:2],
                                                    op=A.add)
                        if "msg" not in SKIP:
                            ae = pw.tile([P, CH], F32)
                            nc.vector.tensor_scalar(
                                out=ae[:], in0=wv_sb[:, cbase:cbase + CH],
                                scalar1=q2_sb[:, 0:1], scalar2=None, op0=A.mult)
                            nc.vector.tensor_tensor(out=asr, in0=asr,
                                                        in1=ae[:].unsqueeze(2),
                                                        op=A.add)
                            e2 = pw.tile([P, CH, 1], F32)
                            nc.scalar.activation(out=e2[:], in_=asr, func=ACT.Exp,
                                                 scale=0.2)
                            nc.scalar.activation(out=asr, in_=asr, func=ACT.Exp)
                            nc.vector.tensor_tensor(out=asr, in0=asr, in1=e2[:],
                                                        op=A.max)
                            gm = G[:, :, 0:HID]
                            p_b = asr.to_broadcast([P, CH, HID])
                            nc.vector.tensor_tensor(out=gm, in0=gm, in1=p_b,
                                                        op=A.mult)
                        if "mm" not in SKIP:
                            oh = poh.tile([P, CH, P], F32)
                            nc.vector.tensor_tensor(
                                out=oh[:],
                                in0=iota_sb[:].unsqueeze(1)
                                    .to_broadcast([P, CH, P]),
                                in1=dstl_sb[:, cbase:cbase + CH]
                                    .unsqueeze(2).to_broadcast([P, CH, P]),
                                op=A.is_equal)
                            for c in range(CH):
                                nc.tensor.matmul(
                                    acc[:], lhsT=oh[:, c, :],
                                    rhs=G[:, c, 0:HID + 1],
                                    start=(hh == 0 and c == 0),
                                    stop=(hh == 1 and c == CH - 1))
                    if "epi" not in SKIP:
                        # epilogue: out2e = [relu(acc/denom + b2) | 1]
                        dn = pw.tile([P, 1], F32)
                        nc.vector.tensor_scalar(out=dn[:], in0=acc[:, HID:HID + 1],
                                                scalar1=1e-16, scalar2=None,
                                                op0=A.add)
                        rc = pw.tile([P, 1], F32)
                        nc.vector.reciprocal(rc[:], dn[:])
                        o2 = pw.tile([P, HID + 1], F32)
                        nc.vector.tensor_scalar(out=o2[:, 0:HID], in0=acc[:, 0:HID],
                                                scalar1=rc[:, 0:1], scalar2=None,
                                                op0=A.mult)
                        nc.vector.tensor_tensor(out=o2[:, 0:HID], in0=o2[:, 0:HID],
                                                in1=b2_sb[:], op=A.add)
                        nc.vector.tensor_scalar(out=o2[:, 0:HID], in0=o2[:, 0:HID],
                                                scalar1=0.0, scalar2=None, op0=A.max)
                        nc.vector.memset(o2[:, HID:HID + 1], 1.0)
                        ohg = poh.tile([P, GPC], F32)
                        nc.vector.tensor_scalar(
                            out=ohg[:], in0=iota_sb[:, 0:GPC],
                            scalar1=gl_sb[:, t:t + 1], scalar2=None, op0=A.is_equal)
                        nc.tensor.matmul(pool_ps[:], lhsT=ohg[:], rhs=o2[:],
                                         start=(t == 0), stop=(t == NT - 1),
                                         skip_group_check=True)

                # ------------- Phase 4: pooled mean + FC ------------------------
                fcw_sb = pc.tile([HID, OUT], F32)
                nc.sync.dma_start(fcw_sb[:], fcw_d[:])
                fcb_sb = pc.tile([P, OUT], F32)
                nc.sync.dma_start(fcb_sb[:], fcb_d[:])
                cnt = pc.tile([GPC, 1], F32)
                nc.vector.tensor_scalar(out=cnt[:], in0=pool_ps[:, HID:HID + 1],
                                        scalar1=1.0, scalar2=None, op0=A.max)
                rcc = pc.tile([GPC, 1], F32)
                nc.vector.reciprocal(rcc[:], cnt[:])
                pooled = pc.tile([GPC, HID], F32)
                nc.vector.tensor_scalar(out=pooled[:], in0=pool_ps[:, 0:HID],
                                        scalar1=rcc[:, 0:1], scalar2=None,
                                        op0=A.mult)
                pT_ps = pps.tile([HID, GPC], F32, space="PSUM")
                nc.tensor.transpose(pT_ps[:], pooled[:], ident[:GPC, :GPC])
                pT = pc.tile([HID, GPC], F32)
                nc.vector.tensor_copy(out=pT[:], in_=pT_ps[:])
                fc_ps = pps.tile([GPC, OUT], F32, space="PSUM")
                nc.tensor.matmul(fc_ps[:], lhsT=pT[:], rhs=fcw_sb[:],
                                 start=True, stop=True)
                res = pc.tile([GPC, OUT], F32)
                nc.vector.tensor_tensor(out=res[:], in0=fc_ps[:],
                                        in1=fcb_sb[:GPC, :], op=A.add)
                nc.sync.dma_start(out_d[:], res[:])

    nc.compile()
    return nc


# ---------------------------------------------------------------------------
# Entry point.
# ---------------------------------------------------------------------------
def run(inputs, cfg, **run_kwargs):
    in_maps, meta = prepare(inputs, cfg)
    nc = build(meta)
    res = run_bass_kernel_spmd(nc, in_maps, core_ids=list(range(NCORES)),
                               **run_kwargs)
    out = np.concatenate([res.results[c]["out"] for c in range(NCORES)],
                         axis=0)
    return np.asarray(out, np.float32), res


def kernel(**inputs) -> np.ndarray:
    out, _ = run(inputs, FULL_CFG)
    return out



# revision 9
# speedup vs baseline: 2.3300x; 2.3300x over previous
"""Trainium2 Bass kernel for a 2-layer edge-featured GAT + mean-pool + FC.

Sharding: 256 graphs split 32-per-core across 8 cores (batch is sorted, so
each core owns a contiguous, graph-aligned node range). Edges live on the
core that owns their destination; per-core node tables are AllGathered
between layers so any core can gather arbitrary source rows.

v2 design (vs the fp32 SWDGE baseline):
- Node tables are bf16. Layer-1 rows are 384 bf16 (768B):
  [h head0(64) | 1 | h1(64) | 1 | h2(64) | 1 | h3(64) | 1 | asrc(4) |
   adst(4) | pad]; the interleaved 1-columns produce softmax denominators
  through the same scatter matmul. Layer-2 rows are 256 bf16 (512B).
- Src-row gathers are SWDGE dma_gather on 4 rotating queues (the Q7
  descriptor generator stalls on ring space with one queue; four queues
  nearly double throughput). Dst-side gathers are gone entirely:
- The edge->dst one-hot matrices (and their transposes) are HOST-PRECOMPUTED
  bf16 constants (the edge structure is input data, not device data).
  oh[128e, 128j] drives the segment-sum scatter matmul; ohT[128j, 128e]
  broadcasts per-dst-node logits to edges via a tiny K=128 matmul
  (adp = ohT^T @ adst_tile).
- p = exp(leaky_relu(asrc+adst+aedge)) is computed per edge (max of two
  exps), multiplied into the message rows including the 1-columns, so one
  matmul per 128-edge chunk accumulates both Sum(p*h) and Sum(p).
  Normalization happens once per node in the epilogue (exactly equivalent
  to the reference's softmax; the max-shift cancels in the ratio).
- Mean-pool one-hot and 1/count are host constants; final FC as in v1.
"""

import sys

sys.path.insert(0, "/opt/trn_rl_repo")

import math
from contextlib import ExitStack

import numpy as np
import ml_dtypes

import concourse.bacc as bacc
import concourse.bass as bass
import concourse.mybir as mybir
import concourse.tile as tile
from concourse.bass_utils import run_bass_kernel_spmd
from concourse.masks import make_identity

P = 128
NCORES = 8
BF = ml_dtypes.bfloat16

FULL_CFG = dict(N=20000, E=640000, FIN=128, HID=64, HEADS=4, NG=256, OUT=32)

F32 = mybir.dt.float32
BF16 = mybir.dt.bfloat16
I16 = mybir.dt.int16

# layer-1 row layout (bf16): 4 x [h(64) | 1] then asrc(4) adst(4) pad -> 384
CW = 65                  # head group width (64 + denom column)
D1R = 4 * CW             # 260
ASRC1, ADST1 = D1R, D1R + 4
ROW1 = 384
# layer-2 row layout (bf16): [h(64) | 1 | asrc(1) | adst(1) | pad] -> 256
ASRC2, ADST2 = CW, CW + 1
ROW2 = 256


# ---------------------------------------------------------------------------
# Host-side preparation: integer index manipulation + array reordering only.
# ---------------------------------------------------------------------------
def prepare(inputs, cfg):
    N, E, FIN, HID, HEADS, NG, OUT = (
        cfg["N"], cfg["E"], cfg["FIN"], cfg["HID"], cfg["HEADS"], cfg["NG"],
        cfg["OUT"],
    )
    GPC = NG // NCORES  # graphs per core

    x = np.asarray(inputs["x"], np.float32)
    ei = np.asarray(inputs["edge_index"], np.int64)
    ea = np.asarray(inputs["edge_attr"], np.float32)
    batch = np.asarray(inputs["batch"], np.int64)
    src, dst = ei[0], ei[1]

    # node ranges per core (graph-aligned; batch is sorted)
    bounds = np.searchsorted(batch, np.arange(NCORES + 1) * GPC)
    node_cnt = np.diff(bounds)
    NT = max(1, math.ceil(node_cnt.max() / P))
    NSLICE = NT * P
    NROWS = NCORES * NSLICE
    assert NROWS < 32768, f"int16 gather index overflow: {NROWS}"

    core_of_node = np.minimum(batch // GPC, NCORES - 1).astype(np.int64)
    rowid = np.empty(N, np.int64)
    for c in range(NCORES):
        ns, ne = bounds[c], bounds[c + 1]
        rowid[ns:ne] = c * NSLICE + np.arange(ne - ns)

    # edges sorted by dst; core blocks are contiguous
    order = np.argsort(dst, kind="stable")
    dsts = dst[order]
    srcs = src[order]
    ws = ea[order, 0]
    ecore = core_of_node[dsts]
    ebounds = np.searchsorted(ecore, np.arange(NCORES + 1))

    # chunks-per-tile: max over all (core, tile), rounded up to even
    cpt_max = 1
    tile_edge_counts = []
    for c in range(NCORES):
        es, ee = ebounds[c], ebounds[c + 1]
        dln = dsts[es:ee] - bounds[c]
        tid = dln // P
        cnts = np.bincount(tid, minlength=NT)
        tile_edge_counts.append(cnts)
        if len(cnts):
            cpt_max = max(cpt_max, math.ceil(cnts.max() / P))
    CPT = cpt_max + (cpt_max % 2)  # even
    CPT = max(CPT, 2)
    CH = CPT // 2
    NCHUNK = NT * CPT

    # per-layer-1/2 attention-edge scalars (tiny float prep, host-replicated)
    q1 = (np.asarray(inputs["We1"], np.float32).reshape(HEADS, HID)
          * np.asarray(inputs["att_edge1"], np.float32)).sum(axis=1)  # [H]
    q2 = float((np.asarray(inputs["We2"], np.float32).reshape(-1)
                * np.asarray(inputs["att_edge2"], np.float32).reshape(-1))
               .sum())

    jj = np.arange(P, dtype=np.int64)

    per_core = []
    for c in range(NCORES):
        ns, ne = bounds[c], bounds[c + 1]
        es, ee = ebounds[c], ebounds[c + 1]
        nloc = ne - ns

        xs = np.zeros((NSLICE, FIN), np.float32)
        xs[:nloc] = x[ns:ne]

        srcrow = np.zeros((NT, CPT * P), np.int64)
        dstl = np.full((NT, CPT * P), -1, np.int64)
        wv = np.zeros((NT, CPT * P), np.float32)

        dln = dsts[es:ee] - ns
        tid = dln // P
        cnts = tile_edge_counts[c]
        off = np.zeros(NT + 1, np.int64)
        off[1:NT + 1] = np.cumsum(cnts[:NT])
        for t in range(NT):
            k = int(cnts[t]) if t < len(cnts) else 0
            if k == 0:
                continue
            sel = slice(es + int(off[t]), es + int(off[t]) + k)
            srcrow[t, :k] = rowid[srcs[sel]]
            dstl[t, :k] = dln[int(off[t]):int(off[t]) + k] % P
            wv[t, :k] = ws[sel]

        # one-hot constants: eq[k, e, j] = (dstl[k*128+e] == j)
        dstl_f = dstl.reshape(NCHUNK, P)
        eq = (dstl_f[:, :, None] == jj[None, None, :])
        oh_dev = np.ascontiguousarray(
            eq.transpose(1, 0, 2).reshape(P, NCHUNK * P)).astype(BF)
        ohT_dev = np.ascontiguousarray(
            eq.transpose(2, 0, 1).reshape(P, NCHUNK * P)).astype(BF)

        # per-edge attention-edge terms (w_e * q_h); pad slots -> 0
        wflat = wv.reshape(NCHUNK, P)
        ae1 = np.ascontiguousarray(
            (wflat[:, :, None] * q1[None, None, :]).transpose(1, 0, 2)
        ).astype(BF)                                   # [128, NCHUNK, H]
        ae2 = np.ascontiguousarray(
            (wflat * q2).transpose(1, 0)).astype(BF)   # [128, NCHUNK]

        def wrap_idx(arr):  # [NT, CPT*P] -> [128, NT*CPT*8] int16
            blocks = []
            for t in range(NT):
                for h in range(2):
                    ids = arr[t, h * CH * P:(h + 1) * CH * P]
                    a = ids.reshape(CH * 8, 16).T  # [16, CH*8]
                    blocks.append(np.tile(a, (8, 1)))
            return np.ascontiguousarray(
                np.concatenate(blocks, axis=1)).astype(np.int16)

        # pooling one-hot + 1/count (host: index data only)
        bl = np.full((NSLICE,), -1, np.int64)
        bl[:nloc] = batch[ns:ne] - c * GPC
        poolg = np.ascontiguousarray(
            (bl.reshape(NT, P)[:, :, None] ==
             np.arange(GPC)[None, None, :]).transpose(1, 0, 2)).astype(BF)
        cnt = np.bincount(bl[:nloc], minlength=GPC).astype(np.float32)
        invc = (1.0 / np.maximum(cnt, 1.0)).reshape(GPC, 1).astype(np.float32)

        per_core.append(dict(
            xs=xs, idxs=wrap_idx(srcrow), oh=oh_dev, ohT=ohT_dev,
            ae1=ae1, ae2=ae2, poolg=poolg, invc=invc,
        ))

    # weight-side constants (tiny, host-replicated)
    W1 = np.asarray(inputs["W1"], np.float32)            # [FIN, H*HID]
    W2 = np.asarray(inputs["W2"], np.float32)            # [H*HID, HID]
    rep = lambda vv: np.tile(np.asarray(vv, np.float32).reshape(1, -1),
                             (P, 1)).copy()
    consts = dict(
        W1=W1,
        W2b=np.ascontiguousarray(
            W2.reshape(2, P, HID).transpose(1, 0, 2)).astype(BF),
        as1b=rep(inputs["att_src1"]), ad1b=rep(inputs["att_dst1"]),
        b1b=rep(inputs["b1"]),
        as2b=rep(inputs["att_src2"]), ad2b=rep(inputs["att_dst2"]),
        b2b=rep(inputs["b2"]),
        fcw=np.asarray(inputs["fcW"], np.float32),
        fcbb=rep(inputs["fcb"]),
    )

    in_maps = []
    for c in range(NCORES):
        m = dict(per_core[c])
        m.update(consts)
        in_maps.append(m)

    meta = dict(NT=NT, CPT=CPT, CH=CH, NSLICE=NSLICE, NROWS=NROWS,
                GPC=GPC, NCHUNK=NCHUNK, **cfg)
    return in_maps, meta


# ---------------------------------------------------------------------------
# Device program.
# ---------------------------------------------------------------------------
def build(meta, reps=1, num_devices=NCORES):
    NT, CPT, CH = meta["NT"], meta["CPT"], meta["CH"]
    NSLICE, NROWS, GPC = meta["NSLICE"], meta["NROWS"], meta["GPC"]
    FIN, HID, HEADS, OUT = meta["FIN"], meta["HID"], meta["HEADS"], meta["OUT"]
    NCHUNK = meta["NCHUNK"]
    D1 = HEADS * HID          # 256
    NI = CH * P               # idxs per gather call
    NIc = NI // 16            # idx columns per call
    A = mybir.AluOpType
    ACT = mybir.ActivationFunctionType
    X = mybir.AxisListType.X
    rg = [list(range(NCORES))]

    nc = bacc.Bacc("TRN2", target_bir_lowering=False, debug=False,
                   num_devices=num_devices,
                   dynamic_dma_scratch_size=65536,
                   num_swdge_queues=4)

    def din(name, shape, dtype=F32):
        return nc.dram_tensor(name, list(shape), dtype,
                              kind="ExternalInput").ap()

    xs = din("xs", (NSLICE, FIN))
    idxs_d = din("idxs", (P, NCHUNK * 8), I16)
    oh_d = din("oh", (P, NCHUNK * P), BF16)
    ohT_d = din("ohT", (P, NCHUNK * P), BF16)
    ae1_d = din("ae1", (P, NCHUNK * HEADS), BF16)
    ae2_d = din("ae2", (P, NCHUNK), BF16)
    poolg_d = din("poolg", (P, NT * GPC), BF16)
    invc_d = din("invc", (GPC, 1))
    W1_d = din("W1", (FIN, D1))
    W2_d = din("W2b", (P, 2 * HID), BF16)
    as1_d = din("as1b", (P, D1))
    ad1_d = din("ad1b", (P, D1))
    b1_d = din("b1b", (P, D1))
    as2_d = din("as2b", (P, HID))
    ad2_d = din("ad2b", (P, HID))
    b2_d = din("b2b", (P, HID))
    fcw_d = din("fcw", (HID, OUT))
    fcb_d = din("fcbb", (P, OUT))

    out_d = nc.dram_tensor("out", [GPC, OUT], F32, kind="ExternalOutput").ap()

    gq = [0]  # rotating SWDGE queue

    with tile.TileContext(nc) as tc, ExitStack() as st:
        constp = st.enter_context(tc.tile_pool(name="constp", bufs=1))
        drp = st.enter_context(tc.tile_pool(name="drp", bufs=1, space="DRAM"))

        identf = constp.tile([P, P], F32)
        make_identity(nc, identf[:])
        ixs_all = constp.tile([P, NCHUNK * 8], I16)
        nc.sync.dma_start(ixs_all[:], idxs_d[:])
        ae1_sb = constp.tile([P, NCHUNK, HEADS], BF16)
        nc.sync.dma_start(ae1_sb[:],
                          ae1_d[:].rearrange("p (k h) -> p k h", h=HEADS))
        ae2_sb = constp.tile([P, NCHUNK], BF16)
        nc.sync.dma_start(ae2_sb[:], ae2_d[:])
        poolg_sb = constp.tile([P, NT, GPC], BF16)
        nc.sync.dma_start(poolg_sb[:],
                          poolg_d[:].rearrange("p (t g) -> p t g", g=GPC))
        invc_sb = constp.tile([GPC, 1], F32)
        nc.sync.dma_start(invc_sb[:], invc_d[:])
        b1_sb = constp.tile([P, D1], F32)
        nc.sync.dma_start(b1_sb[:], b1_d[:])

        for _rep in range(reps):
            t1loc = drp.tile([NSLICE, ROW1], BF16, name=f"t1loc{_rep}")
            t1full = drp.tile([NROWS, ROW1], BF16, addr_space="Shared",
                              name=f"t1full{_rep}")
            t2loc = drp.tile([NSLICE, ROW2], BF16, name=f"t2loc{_rep}")
            t2full = drp.tile([NROWS, ROW2], BF16, addr_space="Shared",
                              name=f"t2full{_rep}")

            adst1_all = constp.tile([P, NT, HEADS], BF16,
                                    name=f"adst1_{_rep}")
            adst2_all = constp.tile([P, NT, 1], BF16, name=f"adst2_{_rep}")
            out1 = constp.tile([P, NT, D1], BF16, name=f"out1_{_rep}")

            # ------------- Phase 0: h1 = x @ W1, logits, table1 -------------
            with tc.tile_pool(name="ph0", bufs=1) as sp, \
                 tc.tile_pool(name="ph0b", bufs=2) as sp2, \
                 tc.tile_pool(name="ph0p", bufs=2, space="PSUM") as pp:
                w1_sb = sp.tile([P, D1], F32)
                nc.sync.dma_start(w1_sb[:], W1_d[:])
                as1_sb = sp.tile([P, D1], F32)
                nc.sync.dma_start(as1_sb[:], as1_d[:])
                ad1_sb = sp.tile([P, D1], F32)
                nc.sync.dma_start(ad1_sb[:], ad1_d[:])
                xall = sp.tile([P, NT, FIN], F32)
                nc.sync.dma_start(xall[:],
                                  xs[:].rearrange("(t p) f -> p t f", p=P))
                for t in range(NT):
                    xT_ps = pp.tile([P, P], F32, space="PSUM")
                    nc.tensor.transpose(xT_ps[:], xall[:, t, :], identf[:])
                    xT = sp2.tile([P, P], F32)
                    nc.vector.tensor_copy(out=xT[:], in_=xT_ps[:])
                    h_ps = pp.tile([P, D1], F32, space="PSUM")
                    nc.tensor.matmul(h_ps[:], lhsT=xT[:], rhs=w1_sb[:],
                                     start=True, stop=True)
                    tmp = sp2.tile([P, D1], F32)
                    red = sp2.tile([P, HEADS], F32)
                    nc.vector.tensor_tensor(out=tmp[:], in0=h_ps[:],
                                            in1=as1_sb[:], op=A.mult)
                    nc.vector.tensor_reduce(
                        out=red[:],
                        in_=tmp[:].rearrange("p (h f) -> p h f", h=HEADS),
                        axis=X, op=A.add)
                    t1t = sp2.tile([P, ROW1], BF16)
                    nc.vector.tensor_copy(out=t1t[:, ASRC1:ASRC1 + HEADS],
                                          in_=red[:])
                    nc.vector.tensor_tensor(out=tmp[:], in0=h_ps[:],
                                            in1=ad1_sb[:], op=A.mult)
                    nc.vector.tensor_reduce(
                        out=red[:],
                        in_=tmp[:].rearrange("p (h f) -> p h f", h=HEADS),
                        axis=X, op=A.add)
                    nc.vector.tensor_copy(out=adst1_all[:, t, :], in_=red[:])
                    nc.vector.tensor_copy(out=t1t[:, ADST1:ADST1 + HEADS],
                                          in_=red[:])
                    hv = t1t[:, 0:D1R].rearrange("p (h f) -> p h f", f=CW)
                    nc.vector.tensor_copy(
                        out=hv[:, :, 0:HID],
                        in_=h_ps[:].rearrange("p (h f) -> p h f", f=HID))
                    nc.vector.memset(hv[:, :, HID:CW], 1.0)
                    nc.vector.memset(t1t[:, ADST1 + HEADS:ROW1], 0.0)
                    nc.sync.dma_start(t1loc[t * P:(t + 1) * P, :], t1t[:])
                nc.gpsimd.collective_compute(
                    "AllGather", A.bypass, replica_groups=rg,
                    ins=[t1loc[:]], outs=[t1full[:]])

            # ------------- Phase 1: layer-1 edge phase ----------------------
            with tc.tile_pool(name="p1g", bufs=3) as pg, \
                 tc.tile_pool(name="p1o", bufs=3) as po, \
                 tc.tile_pool(name="p1w", bufs=2) as pw, \
                 tc.tile_pool(name="p1ps", bufs=2, space="PSUM") as pps, \
                 tc.tile_pool(name="p1pa", bufs=2, space="PSUM") as ppa:
                for t in range(NT):
                    acc = pps.tile([P, D1R], F32, space="PSUM")
                    for hh in range(2):
                        kb = t * CPT + hh * CH       # chunk base
                        gbase = (t * 2 + hh) * NIc
                        G = pg.tile([P, CH, ROW1], BF16)
                        nc.gpsimd.dma_gather(
                            G[:], t1full[:], ixs_all[:, gbase:gbase + NIc],
                            NI, NI, ROW1, single_packet=False,
                            queue_num=gq[0] % 4)
                        gq[0] += 1
                        oh_sb = po.tile([P, CH, P], BF16)
                        nc.sync.dma_start(
                            oh_sb[:], oh_d[:, kb * P:(kb + CH) * P]
                            .rearrange("p (c j) -> p c j", j=P))
                        ohT_sb = po.tile([P, CH, P], BF16)
                        nc.sync.dma_start(
                            ohT_sb[:], ohT_d[:, kb * P:(kb + CH) * P]
                            .rearrange("p (c j) -> p c j", j=P))
                        adp_ps = ppa.tile([P, CH, HEADS], F32, space="PSUM")
                        for c in range(CH):
                            nc.tensor.matmul(adp_ps[:, c, :],
                                             lhsT=ohT_sb[:, c, :],
                                             rhs=adst1_all[:, t, :],
                                             start=True, stop=True)
                        alpha = pw.tile([P, CH, HEADS], F32)
                        nc.vector.tensor_tensor(
                            out=alpha[:], in0=G[:, :, ASRC1:ASRC1 + HEADS],
                            in1=adp_ps[:], op=A.add)
                        nc.vector.tensor_tensor(
                            out=alpha[:], in0=alpha[:],
                            in1=ae1_sb[:, kb:kb + CH, :], op=A.add)
                        e2 = pw.tile([P, CH, HEADS], F32)
                        nc.scalar.activation(out=e2[:], in_=alpha[:],
                                             func=ACT.Exp, scale=0.2)
                        nc.scalar.activation(out=alpha[:], in_=alpha[:],
                                             func=ACT.Exp)
                        p_bf = pw.tile([P, CH, HEADS], BF16)
                        nc.vector.tensor_tensor(out=p_bf[:], in0=alpha[:],
                                                in1=e2[:], op=A.max)
                        gv = G[:, :, 0:D1R].rearrange(
                            "p c (h f) -> p c h f", f=CW)
                        nc.vector.tensor_tensor(
                            out=gv, in0=gv,
                            in1=p_bf[:].unsqueeze(3)
                                .to_broadcast([P, CH, HEADS, CW]),
                            op=A.mult)
                        for c in range(CH):
                            nc.tensor.matmul(
                                acc[:], lhsT=oh_sb[:, c, :],
                                rhs=G[:, c, 0:D1R],
                                start=(hh == 0 and c == 0),
                                stop=(hh == 1 and c == CH - 1))
                    # epilogue: out1 = relu(acc_h / denom_h + b1)
                    accv = acc[:].rearrange("p (h f) -> p h f", f=CW)
                    dn = pw.tile([P, HEADS], F32)
                    nc.vector.tensor_scalar(out=dn[:], in0=accv[:, :, HID],
                                            scalar1=1e-16, scalar2=None,
                                            op0=A.add)
                    rc = pw.tile([P, HEADS], F32)
                    nc.vector.reciprocal(rc[:], dn[:])
                    o1 = pw.tile([P, HEADS, HID], F32)
                    nc.vector.tensor_tensor(
                        out=o1[:], in0=accv[:, :, 0:HID],
                        in1=rc[:].unsqueeze(2).to_broadcast([P, HEADS, HID]),
                        op=A.mult)
                    nc.vector.tensor_tensor(
                        out=o1[:], in0=o1[:],
                        in1=b1_sb[:].rearrange("p (h f) -> p h f", h=HEADS),
                        op=A.add)
                    nc.vector.tensor_scalar(
                        out=out1[:, t, :].rearrange("p (h f) -> p h f",
                                                    h=HEADS),
                        in0=o1[:], scalar1=0.0, scalar2=None, op0=A.max)

            # ------------- Phase 2: h2 = out1 @ W2, table2 ------------------
            with tc.tile_pool(name="ph2", bufs=1) as sp, \
                 tc.tile_pool(name="ph2b", bufs=2) as sp2, \
                 tc.tile_pool(name="ph2p", bufs=2, space="PSUM") as pp:
                identb = sp.tile([P, P], BF16)
                make_identity(nc, identb[:])
                w2_sb = sp.tile([P, 2, HID], BF16)
                nc.sync.dma_start(w2_sb[:],
                                  W2_d[:].rearrange("p (k n) -> p k n", k=2))
                as2_sb = sp.tile([P, HID], F32)
                nc.sync.dma_start(as2_sb[:], as2_d[:])
                ad2_sb = sp.tile([P, HID], F32)
                nc.sync.dma_start(ad2_sb[:], ad2_d[:])
                for t in range(NT):
                    h2_ps = pp.tile([P, HID], F32, space="PSUM")
                    for k in range(2):
                        hT_ps = pp.tile([P, P], BF16, space="PSUM")
                        nc.tensor.transpose(
                            hT_ps[:], out1[:, t, k * P:(k + 1) * P],
                            identb[:])
                        hT = sp2.tile([P, P], BF16)
                        nc.vector.tensor_copy(out=hT[:], in_=hT_ps[:])
                        nc.tensor.matmul(h2_ps[:], lhsT=hT[:],
                                         rhs=w2_sb[:, k, :],
                                         start=(k == 0), stop=(k == 1))
                    t2t = sp2.tile([P, ROW2], BF16)
                    tmp = sp2.tile([P, HID], F32)
                    red1 = sp2.tile([P, 1], F32)
                    nc.vector.tensor_tensor(out=tmp[:], in0=h2_ps[:],
                                            in1=as2_sb[:], op=A.mult)
                    nc.vector.tensor_reduce(out=red1[:],
                                            in_=tmp[:], axis=X, op=A.add)
                    nc.vector.tensor_copy(out=t2t[:, ASRC2:ASRC2 + 1],
                                          in_=red1[:])
                    nc.vector.tensor_tensor(out=tmp[:], in0=h2_ps[:],
                                            in1=ad2_sb[:], op=A.mult)
                    red2 = sp2.tile([P, 1], F32)
                    nc.vector.tensor_reduce(out=red2[:],
                                            in_=tmp[:], axis=X, op=A.add)
                    nc.vector.tensor_copy(out=adst2_all[:, t, :], in_=red2[:])
                    nc.vector.tensor_copy(out=t2t[:, ADST2:ADST2 + 1],
                                          in_=red2[:])
                    nc.vector.tensor_copy(out=t2t[:, 0:HID], in_=h2_ps[:])
                    nc.vector.memset(t2t[:, HID:HID + 1], 1.0)
                    nc.vector.memset(t2t[:, ADST2 + 1:ROW2], 0.0)
                    nc.sync.dma_start(t2loc[t * P:(t + 1) * P, :], t2t[:])
                nc.gpsimd.collective_compute(
                    "AllGather", A.bypass, replica_groups=rg,
                    ins=[t2loc[:]], outs=[t2full[:]])

            # ------------- Phase 3: layer-2 edge phase + pooling ------------
            with tc.tile_pool(name="p3g", bufs=3) as pg, \
                 tc.tile_pool(name="p3o", bufs=3) as po, \
                 tc.tile_pool(name="p3w", bufs=2) as pw, \
                 tc.tile_pool(name="p3c", bufs=1) as pc, \
                 tc.tile_pool(name="p3ps", bufs=2, space="PSUM") as pps, \
                 tc.tile_pool(name="p3pa", bufs=2, space="PSUM") as ppa, \
                 tc.tile_pool(name="p3f", bufs=1, space="PSUM") as ppf, \
                 tc.tile_pool(name="p3pl", bufs=1, space="PSUM") as ppl:
                b2_sb = pc.tile([P, HID], F32)
                nc.sync.dma_start(b2_sb[:], b2_d[:])
                pool_ps = ppl.tile([GPC, HID], F32, space="PSUM")
                for t in range(NT):
                    acc = pps.tile([P, CW], F32, space="PSUM")
                    for hh in range(2):
                        kb = t * CPT + hh * CH
                        gbase = (t * 2 + hh) * NIc
                        G = pg.tile([P, CH, ROW2], BF16)
                        nc.gpsimd.dma_gather(
                            G[:], t2full[:], ixs_all[:, gbase:gbase + NIc],
                            NI, NI, ROW2, single_packet=False,
                            queue_num=gq[0] % 4)
                        gq[0] += 1
                        oh_sb = po.tile([P, CH, P], BF16)
                        nc.sync.dma_start(
                            oh_sb[:], oh_d[:, kb * P:(kb + CH) * P]
                            .rearrange("p (c j) -> p c j", j=P))
                        ohT_sb = po.tile([P, CH, P], BF16)
                        nc.sync.dma_start(
                            ohT_sb[:], ohT_d[:, kb * P:(kb + CH) * P]
                            .rearrange("p (c j) -> p c j", j=P))
                        adp_ps = ppa.tile([P, CH, 1], F32, space="PSUM")
                        for c in range(CH):
                            nc.tensor.matmul(adp_ps[:, c, :],
                                             lhsT=ohT_sb[:, c, :],
                                             rhs=adst2_all[:, t, :],
                                             start=True, stop=True)
                        alpha = pw.tile([P, CH, 1], F32)
                        nc.vector.tensor_tensor(
                            out=alpha[:], in0=G[:, :, ASRC2:ASRC2 + 1],
                            in1=adp_ps[:], op=A.add)
                        nc.vector.tensor_tensor(
                            out=alpha[:], in0=alpha[:],
                            in1=ae2_sb[:, kb:kb + CH].unsqueeze(2),
                            op=A.add)
                        e2 = pw.tile([P, CH, 1], F32)
                        nc.scalar.activation(out=e2[:], in_=alpha[:],
                                             func=ACT.Exp, scale=0.2)
                        nc.scalar.activation(out=alpha[:], in_=alpha[:],
                                             func=ACT.Exp)
                        p_bf = pw.tile([P, CH, 1], BF16)
                        nc.vector.tensor_tensor(out=p_bf[:], in0=alpha[:],
                                                in1=e2[:], op=A.max)
                        gv = G[:, :, 0:CW]
                        nc.vector.tensor_tensor(
                            out=gv, in0=gv,
                            in1=p_bf[:].to_broadcast([P, CH, CW]),
                            op=A.mult)
                        for c in range(CH):
                            nc.tensor.matmul(
                                acc[:], lhsT=oh_sb[:, c, :],
                                rhs=G[:, c, 0:CW],
                                start=(hh == 0 and c == 0),
                                stop=(hh == 1 and c == CH - 1))
                    # epilogue: o2 = relu(acc/denom + b2) -> bf16, pool matmul
                    dn = pw.tile([P, 1], F32)
                    nc.vector.tensor_scalar(out=dn[:], in0=acc[:, HID:CW],
                                            scalar1=1e-16, scalar2=None,
                                            op0=A.add)
                    rc = pw.tile([P, 1], F32)
                    nc.vector.reciprocal(rc[:], dn[:])
                    o2 = pw.tile([P, HID], F32)
                    nc.vector.tensor_scalar(out=o2[:], in0=acc[:, 0:HID],
                                            scalar1=rc[:, 0:1], scalar2=None,
                                            op0=A.mult)
                    nc.vector.tensor_tensor(out=o2[:], in0=o2[:],
                                            in1=b2_sb[:], op=A.add)
                    o2b = pw.tile([P, HID], BF16)
                    nc.vector.tensor_scalar(out=o2b[:], in0=o2[:],
                                            scalar1=0.0, scalar2=None,
                                            op0=A.max)
                    nc.tensor.matmul(pool_ps[:], lhsT=poolg_sb[:, t, :],
                                     rhs=o2b[:], start=(t == 0),
                                     stop=(t == NT - 1),
                                     skip_group_check=True)

                # ------------- Phase 4: pooled mean + FC --------------------
                fcw_sb = pc.tile([HID, OUT], F32)
                nc.sync.dma_start(fcw_sb[:], fcw_d[:])
                fcb_sb = pc.tile([P, OUT], F32)
                nc.sync.dma_start(fcb_sb[:], fcb_d[:])
                pooled = pc.tile([GPC, HID], F32)
                nc.vector.tensor_scalar(out=pooled[:], in0=pool_ps[:],
                                        scalar1=invc_sb[:, 0:1], scalar2=None,
                                        op0=A.mult)
                pT_ps = ppf.tile([HID, GPC], F32, space="PSUM")
                nc.tensor.transpose(pT_ps[:], pooled[:], identf[:GPC, :GPC])
                pT = pc.tile([HID, GPC], F32)
                nc.vector.tensor_copy(out=pT[:], in_=pT_ps[:])
                fc_ps = ppf.tile([GPC, OUT], F32, space="PSUM")
                nc.tensor.matmul(fc_ps[:], lhsT=pT[:], rhs=fcw_sb[:],
                                 start=True, stop=True)
                res = pc.tile([GPC, OUT], F32)
                nc.vector.tensor_tensor(out=res[:], in0=fc_ps[:],
                                        in1=fcb_sb[:GPC, :], op=A.add)
                nc.sync.dma_start(out_d[:], res[:])

    nc.compile()
    return nc


# ---------------------------------------------------------------------------
# Entry point.
# ---------------------------------------------------------------------------
def run(inputs, cfg, **run_kwargs):
    in_maps, meta = prepare(inputs, cfg)
    nc = build(meta)
    res = run_bass_kernel_spmd(nc, in_maps, core_ids=list(range(NCORES)),
                               **run_kwargs)
    out = np.concatenate([res.results[c]["out"] for c in range(NCORES)],
                         axis=0)
    return np.asarray(out, np.float32), res


def kernel(**inputs) -> np.ndarray:
    out, _ = run(inputs, FULL_CFG)
    return out
